# revision 18
# baseline (speedup 1.0000x reference)
"""Causal multi-head attention (prefill) on 8 Trainium2 NeuronCores.

Problem: x[2,2048,1024], Wq/Wk/Wv/Wo[1024,1024] (torch Linear [out,in]),
bo[1024]; y = MHA(x) with 16 heads of dim 64, causal softmax.

Sharding (data + tensor parallel): core c handles batch b=c//4 and head
group g=c%4 (4 heads = rows [256g, 256g+256) of Wq/Wk/Wv, cols of Wo).
Each core computes a partial y contribution through its Wo column slice;
the host sums the 4 partials per batch and adds bo.

Host-side prep (off the device critical path): x and the weight slices
are pre-transposed to contraction-major layout and cast to bf16, so the
device kernel does zero layout work. All matmuls run bf16 with f32 PSUM
accumulation.

Per-core kernel, software-pipelined emission (one body = one iteration
of the hardware timing loop; weights/constants load once outside it):
  - Q^T/K^T d-major per head pair (h_even in partitions 0:64, h_odd in
    64:128); V t-major with a ones column ([V|1]) so the PV matmul
    yields the softmax denominator in row 64 for free.
  - Causality: for the diagonal 128x128 sub-block the mask is ADDED in
    PSUM by an extra matmul pass (stationary -240*I, moving
    strict-upper-triangular ones) before the exp, so masked entries
    exp to 0. Off-diagonal k-tiles are fully valid.
  - Softmax k-major without max subtraction (scores/8 bounded ~|3|).
    ACT exp reads S^T from PSUM and writes bf16 P for the PV matmul.
    The attention inner loop is ACT-paced (~1.06us/tile vs ~0.85us PE),
    so the emission interleaves "filler" PE work — this chunk's own K/V
    (first needed at the diagonal), the next chunk's Q, the previous
    chunk's output projection — between attention tiles, with the PV of
    tile k emitted after the S of tile k+2 to hide the exp latency.
  - PSUM: a double-buffered 2-bank "st" ring for S tiles (and for the
    pipelined start/end-of-body projection bursts), a 1-bank "pp" ring
    for paced fillers, and a 3-deep 1-bank "po" ring for the per-head
    PV accumulators. gpsimd never touches PSUM (hardware restriction).
  - Normalization: reciprocal of denominators (DVE), gpsimd
    partition_broadcast, one multiply per head writes normalized out^T
    (d-major bf16) — the stationary layout the output projection needs.
  - The body is rotated: x chunk 0 and Q/K/V of chunk 0 for the NEXT
    iteration are produced at the END of the body, interleaved with the
    last chunk's output projection, so the PE stays busy through the
    final norm latency and the next iteration starts hot. A preamble
    outside the loop primes chunk 0 for the first iteration.
"""

import numpy as np

import concourse.bass as bass
import concourse.mybir as mybir
import concourse.tile as tile
from concourse import bacc
from concourse.bass_utils import run_bass_kernel_spmd

P = 128
C = 1024
HD = 64
HPC = 4  # heads per core
NPAIR = 2  # head pairs per core
CS = C // P  # 8 c-subtiles
DS = HPC * HD  # 256, d-slice of this core
QB = 512  # q-block (PSUM bank pair width in fp32)
T_FULL = 2048
N_CORES = 8

f32 = mybir.dt.float32
bf16 = mybir.dt.bfloat16
AF = mybir.ActivationFunctionType

LABELS = {}


def _L(inst, txt):
    try:
        LABELS[inst.ins.name] = txt
    except Exception:
        pass
    return inst


class _Sched:
    """Duration-paced filler queue: PE work units emitted between
    attention tiles. Entries carry an optional deadline tile index by
    which they must have been emitted (consumer dependency)."""

    def __init__(self):
        self.q = []
        self.debt = 0.0

    def add(self, dur, fn, deadline=None):
        self.q.append((dur, fn, deadline))

    def pace(self, quota, tile=None):
        """Emit fillers until ~quota ns of PE work has been issued; always
        emit entries whose deadline is within 3 tiles."""
        self.debt += quota
        npop = 0
        while self.q and npop < 2:
            dur, fn, dl = self.q[0]
            urgent = tile is not None and dl is not None and dl <= tile + 3
            if not urgent and (self.debt < dur * 0.5 or npop >= 1):
                break
            self.q.pop(0)
            fn()
            self.debt -= dur
            npop += 1

    def drain(self):
        for dur, fn, dl in self.q:
            fn()
        self.q = []
        self.debt = 0.0


QK_NS, V_NS, Y_NS = 1707.0, 853.0, 427.0


def _make_emitters(nc, tl, pools, y_d):
    wqT, wkT, wvT, woT = tl["wqT"], tl["wkT"], tl["wvT"], tl["woT"]
    xT, qT, kT, vE, outT = tl["xT"], tl["qT"], tl["kT"], tl["vE"], tl["outT"]
    negi, utri = tl["negi"], tl["utri"]
    stp, ppp, po, sbP, sb_norm, sb_y = pools

    PH = ["?"]

    def _MM(*a, **kw):
        return _L(nc.tensor.matmul(*a, **kw), PH[0])

    def proj_tile(ring):
        """1-bank accumulator: paced fillers use the small pp ring so they
        never contend with the attention st ring; explicit start/end-of-
        body projection bursts use the idle st ring, which is double-
        buffered so back-to-back groups pipeline."""
        if ring == "st":
            return stp.tile([P, 2, QB], f32, tag="st", name="fill")[:, 0, :]
        return ppp.tile([P, QB], f32, tag="pp", name="fill")[:]

    def emit_qk(jc, pr, which, ring="pp"):
        """One Q^T or K^T projection group: 8 matmuls + copy."""
        PH[0] = f"qk{which}_jc{jc}_pr{pr}"
        q0 = jc * QB
        wT = wqT if which == "q" else wkT
        dstT = (qT if which == "q" else kT)[pr]
        pp = proj_tile(ring)
        for cs in range(CS):
            _MM(
                pp,
                wT[:, cs, pr * P : (pr + 1) * P],
                xT[:, cs, q0 : q0 + QB],
                start=(cs == 0),
                stop=(cs == CS - 1),
            )
        nc.vector.tensor_copy(dstT[:, q0 : q0 + QB], pp)

    def emit_v(jc, ol, ring="pp"):
        """One V projection group (one t-tile): 8 matmuls + copy."""
        PH[0] = f"v_jc{jc}_ol{ol}"
        tt = jc * (QB // P) + ol
        vp = proj_tile(ring)
        for cs in range(CS):
            _MM(
                vp[:, 0:DS],
                xT[:, cs, tt * P : (tt + 1) * P],
                wvT[:, cs, :],
                start=(cs == 0),
                stop=(cs == CS - 1),
            )
        nc.vector.tensor_copy(
            vE[:, tt, :, 0:HD],
            vp[:, 0:DS].rearrange("p (h d) -> p h d", h=HPC),
        )

    def emit_y(jc, ol, doc, ring="pp", copy_eng="dve"):
        """One output-projection unit: 2 matmuls + copy + store."""
        PH[0] = f"y_jc{jc}_ol{ol}_doc{doc}"
        tt = jc * (QB // P) + ol
        yp = proj_tile(ring)
        for pr in range(NPAIR):
            _MM(
                yp,
                outT[pr][:, tt * P : (tt + 1) * P],
                woT[:, pr, doc * QB : (doc + 1) * QB],
                start=(pr == 0),
                stop=(pr == NPAIR - 1),
            )
        yv = sb_y.tile([P, QB], bf16, tag="yv")
        if copy_eng == "act":
            nc.scalar.copy(yv[:], yp)
        else:
            # gpsimd cannot read PSUM, so the other copies go to DVE
            nc.vector.tensor_copy(yv[:], yp)
        nc.sync.dma_start(
            y_d[tt * P : (tt + 1) * P, doc * QB : (doc + 1) * QB], yv[:]
        )

    def emit_s(jc, pr, kt):
        """S^T for one k-tile (both heads of the pair), causal-masked."""
        PH[0] = f"S_jc{jc}_pr{pr}_kt{kt}"
        q0 = jc * QB
        s = kt - 4 * jc
        qoff = max(s, 0) * P
        st_ = stp.tile([P, 2, QB], f32, tag="st", name="st_")
        for hi in range(2):
            hsel = slice(hi * HD, (hi + 1) * HD)
            if s >= 0:
                _MM(
                    st_[:, hi, qoff : qoff + P],
                    negi,
                    utri,
                    start=True,
                    stop=False,
                )
                _MM(
                    st_[:, hi, qoff : qoff + P],
                    kT[pr][hsel, kt * P : (kt + 1) * P],
                    qT[pr][hsel, q0 + qoff : q0 + qoff + P],
                    start=False,
                    stop=True,
                    tile_position=(hi * HD, 0),
                )
                if qoff + P < QB:
                    _MM(
                        st_[:, hi, qoff + P : QB],
                        kT[pr][hsel, kt * P : (kt + 1) * P],
                        qT[pr][hsel, q0 + qoff + P : q0 + QB],
                        start=True,
                        stop=True,
                        tile_position=(hi * HD, 0),
                    )
            else:
                _MM(
                    st_[:, hi, :],
                    kT[pr][hsel, kt * P : (kt + 1) * P],
                    qT[pr][hsel, q0 : q0 + QB],
                    start=True,
                    stop=True,
                    tile_position=(hi * HD, 0),
                )
        pt = sbP.tile([P, 2, QB], bf16, tag="pT")
        _L(
            nc.scalar.activation(
                pt[:, :, qoff:QB], st_[:, :, qoff:QB], AF.Exp, scale=0.125
            ),
            f"exp_jc{jc}_pr{pr}_kt{kt}",
        )
        return pt, qoff

    def emit_pv(jc, pr, kt, nkt, oext, pt, qoff):
        PH[0] = f"PV_jc{jc}_pr{pr}_kt{kt}"
        for hi in range(2):
            h = pr * 2 + hi
            _MM(
                oext[hi][:, qoff:QB],
                vE[:, kt, h, :],
                pt[:, hi, qoff:QB],
                start=(kt == 0),
                stop=(kt == nkt - 1),
            )

    def emit_norm(jc, pr, oexts):
        # both recips, then both broadcasts, then both mults, so the
        # DVE and Pool chains for the two heads overlap
        q0 = jc * QB
        rcs, rss = [], []
        for hi in range(2):
            rc = sb_norm.tile([1, QB], f32, tag=f"rc{hi}", name=f"rc{hi}")
            nc.vector.reciprocal(rc[:], oexts[pr][hi][HD : HD + 1, :])
            rcs.append(rc)
        for hi in range(2):
            rs = sb_norm.tile([HD, QB], f32, tag=f"rs{hi}", name=f"rs{hi}")
            nc.gpsimd.partition_broadcast(rs[:], rcs[hi][:], channels=HD)
            rss.append(rs)
        for hi in range(2):
            nc.vector.tensor_tensor(
                outT[pr][hi * HD : (hi + 1) * HD, q0 : q0 + QB],
                oexts[pr][hi][0:HD, :],
                rss[hi][:],
                mybir.AluOpType.mult,
            )

    def oext_pair():
        return [
            po.tile([HD + 1, QB], f32, tag="po", name=f"oext{hi}")
            for hi in range(2)
        ]

    return dict(
        qk=emit_qk, v=emit_v, y=emit_y, s=emit_s, pv=emit_pv,
        norm=emit_norm, oext=oext_pair,
    )


def _body(nc, NQB, E, dma_x_chunk):
    """One timing-loop iteration. Chunk 0's x / Q / K / V for the NEXT
    iteration are produced at the end, interleaved with the last chunk's
    output projection."""
    for jc in range(1, NQB):
        dma_x_chunk(jc)

    oexts = {}
    for jc in range(NQB):
        sched = _Sched()
        # this chunk's own K/V, first consumed at the diagonal (kt=4jc)
        if jc >= 1:
            for pr in range(NPAIR):
                sched.add(
                    QK_NS,
                    lambda jc=jc, pr=pr: E["qk"](jc, pr, "k"),
                    deadline=4 * jc - 4,
                )
            for ol in range(QB // P):
                sched.add(
                    V_NS,
                    lambda jc=jc, ol=ol: E["v"](jc, ol),
                    deadline=4 * jc - 2,
                )
        # next chunk's Q, consumed from its first tile
        if jc + 1 < NQB:
            for pr in range(NPAIR):
                sched.add(QK_NS, lambda jc=jc, pr=pr: E["qk"](jc + 1, pr, "q"))
        # previous chunk's output projection
        if jc >= 1:
            for ol in range(QB // P):
                for doc in range(C // QB):
                    sched.add(
                        Y_NS,
                        lambda jc=jc, ol=ol, doc=doc: E["y"](jc - 1, ol, doc),
                    )

        nkt = 4 * jc + 4
        ntiles = NPAIR * nkt
        quota = sum(d for d, _, _ in sched.q) / ntiles
        tile_i = 0

        for pr in range(NPAIR):
            oexts[pr] = E["oext"]()
            # 2-deep software pipeline: PV of tile k goes out after the
            # S of tile k+2, so the exp latency never stalls the PE
            pend = []
            for kt in range(nkt):
                pt, qoff = E["s"](jc, pr, kt)
                pend.append((kt, pt, qoff))
                if kt < 2:
                    # extra pacing while the exp pipeline fills and the
                    # previous pair's norm drains the oext ring
                    sched.pace(quota, tile_i)
                if len(pend) > 2:
                    if kt == 2:
                        sched.pace(quota, tile_i)
                    k0, p0, o0 = pend.pop(0)
                    E["pv"](jc, pr, k0, nkt, oexts[pr], p0, o0)
                    sched.pace(quota, tile_i)
                tile_i += 1
            if jc == NQB - 1 and pr == NPAIR - 1:
                # keep the very tail tight: nothing between the last
                # PVs and the norm that gates the final y projection
                sched.drain()
            for k0, p0, o0 in pend:
                E["pv"](jc, pr, k0, nkt, oexts[pr], p0, o0)
            if pr == NPAIR - 1:
                # drain leftovers before the last norm so their copies
                # are not queued behind the norm chain on DVE/Pool
                sched.drain()
            E["norm"](jc, pr, oexts)

    # tail: the last chunk's output projection interleaved with the NEXT
    # iteration's chunk-0 projections — the projections have no pending
    # dependencies, so the PE stays busy through the final norm latency
    nxt = []
    for pr in range(NPAIR):
        nxt.append(lambda pr=pr: E["qk"](0, pr, "q", ring="st"))
        nxt.append(lambda pr=pr: E["qk"](0, pr, "k", ring="st"))
    for ol in range(QB // P):
        nxt.append(lambda ol=ol: E["v"](0, ol, ring="st"))
    yu = [(ol, doc) for ol in range(QB // P) for doc in range(C // QB)]
    for i in range(max(len(nxt), len(yu))):
        if i < len(nxt):
            nxt[i]()
        if i < len(yu):
            ol, doc = yu[i]
            E["y"](NQB - 1, ol, doc, ring="pp", copy_eng="act")

    # prefetch x chunk 0 for the next iteration (after its last readers)
    dma_x_chunk(0)


def build_core_kernel(nc, tc, T, iters=1):
    TO = T // P  # t-tiles
    NQB = T // QB  # q-blocks / chunks

    xt_d = nc.dram_tensor("xt", [C, T], bf16, kind="ExternalInput").ap()
    wq_d = nc.dram_tensor("wq", [C, DS], bf16, kind="ExternalInput").ap()
    wk_d = nc.dram_tensor("wk", [C, DS], bf16, kind="ExternalInput").ap()
    wv_d = nc.dram_tensor("wv", [C, DS], bf16, kind="ExternalInput").ap()
    wo_d = nc.dram_tensor("wo", [DS, C], bf16, kind="ExternalInput").ap()
    negi_d = nc.dram_tensor("negi", [P, P], bf16, kind="ExternalInput").ap()
    utri_d = nc.dram_tensor("utri", [P, P], bf16, kind="ExternalInput").ap()
    y_d = nc.dram_tensor("y", [T, C], bf16, kind="ExternalOutput").ap()

    import contextlib

    stack = contextlib.ExitStack()
    persist = stack.enter_context(tc.tile_pool(name="persist", bufs=1))

    negi = persist.tile([P, P], bf16, tag="negi")
    utri = persist.tile([P, P], bf16, tag="utri")
    nc.sync.dma_start(negi[:], negi_d)
    nc.sync.dma_start(utri[:], utri_d)

    wqT = persist.tile([P, CS, DS], bf16, tag="wqT")
    wkT = persist.tile([P, CS, DS], bf16, tag="wkT")
    wvT = persist.tile([P, CS, DS], bf16, tag="wvT")
    for w_d, w_t in ((wq_d, wqT), (wk_d, wkT), (wv_d, wvT)):
        nc.sync.dma_start(w_t[:], w_d.rearrange("(o p) d -> p o d", p=P))
    woT = persist.tile([P, NPAIR, C], bf16, tag="woT")
    nc.sync.dma_start(woT[:], wo_d.rearrange("(o p) d -> p o d", p=P))

    xT = persist.tile([P, CS, T], bf16, tag="xT")
    qT = [persist.tile([P, T], bf16, tag=f"qT{p}", name=f"qT{p}") for p in range(NPAIR)]
    kT = [persist.tile([P, T], bf16, tag=f"kT{p}", name=f"kT{p}") for p in range(NPAIR)]
    vE = persist.tile([P, TO, HPC, HD + 1], bf16, tag="vE")
    outT = [
        persist.tile([P, T], bf16, tag=f"outT{p}", name=f"outT{p}")
        for p in range(NPAIR)
    ]
    nc.gpsimd.memset(vE[:, :, :, HD : HD + 1], 1.0)

    def dma_x_chunk(jc):
        q0 = jc * QB
        nc.sync.dma_start(
            xT[:, :, q0 : q0 + QB],
            xt_d[:, q0 : q0 + QB].rearrange("(o p) t -> p o t", p=P),
        )

    tl = dict(
        negi=negi, utri=utri, wqT=wqT, wkT=wkT, wvT=wvT, woT=woT,
        xT=xT, qT=qT, kT=kT, vE=vE, outT=outT,
    )

    pools = (
        stack.enter_context(tc.tile_pool(name="st", bufs=2, space="PSUM")),
        stack.enter_context(tc.tile_pool(name="pp", bufs=1, space="PSUM")),
        stack.enter_context(tc.tile_pool(name="po", bufs=3, space="PSUM")),
        stack.enter_context(tc.tile_pool(name="sbP", bufs=4)),
        stack.enter_context(tc.tile_pool(name="sb_norm", bufs=2)),
        stack.enter_context(tc.tile_pool(name="sb_y", bufs=4)),
    )
    E = _make_emitters(nc, tl, pools, y_d)

    # preamble: prime chunk 0 (x + projections) for the first iteration
    dma_x_chunk(0)
    for pr in range(NPAIR):
        E["qk"](0, pr, "q", ring="st")
        E["qk"](0, pr, "k", ring="st")
    for ol in range(QB // P):
        E["v"](0, ol, ring="st")

    if iters < 0:
        # profiling aid: python-unrolled bodies (no hardware loop)
        for _ in range(-iters):
            _body(nc, NQB, E, dma_x_chunk)
    else:
        loop_cm = (
            tc.For_i(0, iters, 1) if iters > 1 else contextlib.nullcontext()
        )
        with loop_cm:
            _body(nc, NQB, E, dma_x_chunk)

    stack.close()


def build_nc(T=T_FULL, iters=1):
    nc = bacc.Bacc("TRN2", target_bir_lowering=False, debug=False, num_devices=N_CORES)
    with tile.TileContext(nc) as tc:
        build_core_kernel(nc, tc, T, iters=iters)
    nc.compile()
    return nc


def make_consts():
    import ml_dtypes

    negi = (-240.0 * np.eye(P)).astype(ml_dtypes.bfloat16)
    k = np.arange(P)
    # utri[k, q] = 1 where masked (q < k) in S^T[k, q]
    utri = (k[None, :] < k[:, None]).astype(ml_dtypes.bfloat16)
    return negi, utri


def make_in_maps(x, Wq, Wk, Wv, Wo):
    """Per-core input dicts. Core c: batch c//4, head group c%4."""
    import ml_dtypes

    bf = ml_dtypes.bfloat16
    negi, utri = make_consts()
    in_maps = []
    for c in range(N_CORES):
        b, g = divmod(c, 4)
        ds = slice(g * DS, (g + 1) * DS)
        in_maps.append(
            {
                "xt": np.ascontiguousarray(x[b].T).astype(bf),
                "wq": np.ascontiguousarray(Wq[ds, :].T).astype(bf),
                "wk": np.ascontiguousarray(Wk[ds, :].T).astype(bf),
                "wv": np.ascontiguousarray(Wv[ds, :].T).astype(bf),
                "wo": np.ascontiguousarray(Wo[:, ds].T).astype(bf),
                "negi": negi,
                "utri": utri,
            }
        )
    return in_maps


def gather(results, bo):
    """Sum partial outputs per batch, add bias."""
    B = N_CORES // 4
    y = np.zeros((B, T_FULL, C), dtype=np.float64)
    for c in range(N_CORES):
        y[c // 4] += np.asarray(results[c]["y"], dtype=np.float64)
    y += bo.astype(np.float64)
    return y.astype(np.float32)


_NC_CACHE = {}


def get_nc():
    if "nc" not in _NC_CACHE:
        _NC_CACHE["nc"] = build_nc()
    return _NC_CACHE["nc"]


def kernel(x, Wq, Wk, Wv, Wo, bo):
    x = np.asarray(x, dtype=np.float32)
    Wq = np.asarray(Wq, dtype=np.float32)
    Wk = np.asarray(Wk, dtype=np.float32)
    Wv = np.asarray(Wv, dtype=np.float32)
    Wo = np.asarray(Wo, dtype=np.float32)
    bo = np.asarray(bo, dtype=np.float32)
    nc = get_nc()
    in_maps = make_in_maps(x, Wq, Wk, Wv, Wo)
    res = run_bass_kernel_spmd(nc, in_maps, core_ids=list(range(N_CORES)))
    return gather(res.results, bo)


# revision 19
# speedup vs baseline: 1.0530x; 1.0530x over previous
"""Causal multi-head attention (prefill) on 8 Trainium2 NeuronCores.

Problem: x[2,2048,1024], Wq/Wk/Wv/Wo[1024,1024] (torch Linear [out,in]),
bo[1024]; y = MHA(x) with 16 heads of dim 64, causal softmax.

Sharding (data + tensor parallel): core c handles batch b=c//4 and head
group g=c%4 (4 heads = rows [256g, 256g+256) of Wq/Wk/Wv, cols of Wo).
Each core computes a partial y contribution through its Wo column slice;
the host sums the 4 partials per batch and adds bo.

Host-side prep (off the device critical path): x and the weight slices
are pre-transposed to contraction-major layout and cast to bf16, so the
device kernel does zero layout work. All matmuls run bf16 with f32 PSUM
accumulation.

Per-core kernel, software-pipelined emission (one body = one iteration
of the hardware timing loop; weights/constants load once outside it):
  - Q^T/K^T d-major per head pair (h_even in partitions 0:64, h_odd in
    64:128); V t-major with a ones column ([V|1]) so the PV matmul
    yields the softmax denominator in row 64 for free.
  - Causality: for the diagonal 128x128 sub-block the mask is ADDED in
    PSUM by an extra matmul pass (stationary -240*I, moving
    strict-upper-triangular ones) before the exp, so masked entries
    exp to 0. Off-diagonal k-tiles are fully valid.
  - Softmax k-major without max subtraction (scores/8 bounded ~|3|).
    ACT exp reads S^T from PSUM and writes bf16 P for the PV matmul.
    The attention inner loop is ACT-paced (~1.06us/tile vs ~0.85us PE),
    so the emission interleaves "filler" PE work — this chunk's own K/V
    (first needed at the diagonal), the next chunk's Q, the previous
    chunk's output projection — between attention tiles, with the PV of
    tile k emitted after the S of tile k+2 to hide the exp latency.
  - PSUM: a double-buffered 2-bank "st" ring for S tiles (and for the
    pipelined start/end-of-body projection bursts), a 1-bank "pp" ring
    for paced fillers, and a 3-deep 1-bank "po" ring for the per-head
    PV accumulators. gpsimd never touches PSUM (hardware restriction).
  - Normalization: reciprocal of denominators (DVE), gpsimd
    partition_broadcast, one multiply per head writes normalized out^T
    (d-major bf16) — the stationary layout the output projection needs.
  - The body is rotated: x chunk 0 and Q/K/V of chunk 0 for the NEXT
    iteration are produced at the END of the body, interleaved with the
    last chunk's output projection, so the PE stays busy through the
    final norm latency and the next iteration starts hot. A preamble
    outside the loop primes chunk 0 for the first iteration.
"""

import numpy as np

import concourse.bass as bass
import concourse.mybir as mybir
import concourse.tile as tile
from concourse import bacc
from concourse.bass_utils import run_bass_kernel_spmd

P = 128
C = 1024
HD = 64
HPC = 4  # heads per core
NPAIR = 2  # head pairs per core
CS = C // P  # 8 c-subtiles
DS = HPC * HD  # 256, d-slice of this core
QB = 512  # q-block (PSUM bank pair width in fp32)
T_FULL = 2048
N_CORES = 8

f32 = mybir.dt.float32
bf16 = mybir.dt.bfloat16
AF = mybir.ActivationFunctionType

LABELS = {}


def _L(inst, txt):
    try:
        LABELS[inst.ins.name] = txt
    except Exception:
        pass
    return inst


class _Sched:
    """Duration-paced filler queue: PE work units emitted between
    attention tiles. Entries carry an optional deadline tile index by
    which they must have been emitted (consumer dependency)."""

    def __init__(self):
        self.q = []
        self.debt = 0.0

    def add(self, dur, fn, deadline=None):
        self.q.append((dur, fn, deadline))

    def pace(self, quota, tile=None):
        """Emit fillers until ~quota ns of PE work has been issued; always
        emit entries whose deadline is within 3 tiles."""
        self.debt += quota
        npop = 0
        while self.q and npop < 2:
            dur, fn, dl = self.q[0]
            urgent = tile is not None and dl is not None and dl <= tile + 3
            if not urgent and (self.debt < dur * 0.5 or npop >= 1):
                break
            self.q.pop(0)
            fn()
            self.debt -= dur
            npop += 1

    def drain(self):
        for dur, fn, dl in self.q:
            fn()
        self.q = []
        self.debt = 0.0


QK_NS, V_NS, Y_NS = 1707.0, 853.0, 427.0


def _make_emitters(nc, tl, pools, y_d):
    wqT, wkT, wvT, woT = tl["wqT"], tl["wkT"], tl["wvT"], tl["woT"]
    xT, qT, kT, vE, outT = tl["xT"], tl["qT"], tl["kT"], tl["vE"], tl["outT"]
    negi, utri = tl["negi"], tl["utri"]
    stp, ppp, po, sbP, sb_norm, sb_y = pools

    PH = ["?"]

    def _MM(*a, **kw):
        return _L(nc.tensor.matmul(*a, **kw), PH[0])

    def proj_tile(ring):
        """1-bank accumulator: paced fillers use the small pp ring so they
        never contend with the attention st ring; explicit start/end-of-
        body projection bursts use the idle st ring, which is double-
        buffered so back-to-back groups pipeline."""
        if ring == "st":
            return stp.tile([P, 2, QB], f32, tag="st", name="fill")[:, 0, :]
        return ppp.tile([P, QB], f32, tag="pp", name="fill")[:]

    def emit_qk(jc, pr, which, ring="pp"):
        """One Q^T or K^T projection group: 8 matmuls + copy."""
        PH[0] = f"qk{which}_jc{jc}_pr{pr}"
        q0 = jc * QB
        wT = wqT if which == "q" else wkT
        dstT = (qT if which == "q" else kT)[pr]
        pp = proj_tile(ring)
        for cs in range(CS):
            _MM(
                pp,
                wT[:, cs, pr * P : (pr + 1) * P],
                xT[:, cs, q0 : q0 + QB],
                start=(cs == 0),
                stop=(cs == CS - 1),
            )
        nc.vector.tensor_copy(dstT[:, q0 : q0 + QB], pp)

    def emit_v(jc, ol, ring="pp"):
        """One V projection group (one t-tile): 8 matmuls + copy."""
        PH[0] = f"v_jc{jc}_ol{ol}"
        tt = jc * (QB // P) + ol
        vp = proj_tile(ring)
        for cs in range(CS):
            _MM(
                vp[:, 0:DS],
                xT[:, cs, tt * P : (tt + 1) * P],
                wvT[:, cs, :],
                start=(cs == 0),
                stop=(cs == CS - 1),
            )
        nc.vector.tensor_copy(
            vE[:, tt, :, 0:HD],
            vp[:, 0:DS].rearrange("p (h d) -> p h d", h=HPC),
        )

    def emit_y(jc, ol, doc, ring="pp", copy_eng="dve"):
        """One output-projection unit: 2 matmuls + copy + store."""
        PH[0] = f"y_jc{jc}_ol{ol}_doc{doc}"
        tt = jc * (QB // P) + ol
        yp = proj_tile(ring)
        for pr in range(NPAIR):
            _MM(
                yp,
                outT[pr][:, tt * P : (tt + 1) * P],
                woT[:, pr, doc * QB : (doc + 1) * QB],
                start=(pr == 0),
                stop=(pr == NPAIR - 1),
            )
        yv = sb_y.tile([P, QB], f32, tag="yv")
        if copy_eng == "act":
            nc.scalar.copy(yv[:], yp)
        else:
            # gpsimd cannot read PSUM, so the other copies go to DVE
            nc.vector.tensor_copy(yv[:], yp)
        nc.sync.dma_start(
            y_d[tt * P : (tt + 1) * P, doc * QB : (doc + 1) * QB], yv[:]
        )

    def emit_s(jc, pr, kt):
        """S^T for one k-tile (both heads of the pair), causal-masked."""
        PH[0] = f"S_jc{jc}_pr{pr}_kt{kt}"
        q0 = jc * QB
        s = kt - 4 * jc
        qoff = max(s, 0) * P
        st_ = stp.tile([P, 2, QB], f32, tag="st", name="st_")
        for hi in range(2):
            hsel = slice(hi * HD, (hi + 1) * HD)
            if s >= 0:
                _MM(
                    st_[:, hi, qoff : qoff + P],
                    negi,
                    utri,
                    start=True,
                    stop=False,
                )
                _MM(
                    st_[:, hi, qoff : qoff + P],
                    kT[pr][hsel, kt * P : (kt + 1) * P],
                    qT[pr][hsel, q0 + qoff : q0 + qoff + P],
                    start=False,
                    stop=True,
                    tile_position=(hi * HD, 0),
                )
                if qoff + P < QB:
                    _MM(
                        st_[:, hi, qoff + P : QB],
                        kT[pr][hsel, kt * P : (kt + 1) * P],
                        qT[pr][hsel, q0 + qoff + P : q0 + QB],
                        start=True,
                        stop=True,
                        tile_position=(hi * HD, 0),
                    )
            else:
                _MM(
                    st_[:, hi, :],
                    kT[pr][hsel, kt * P : (kt + 1) * P],
                    qT[pr][hsel, q0 : q0 + QB],
                    start=True,
                    stop=True,
                    tile_position=(hi * HD, 0),
                )
        pt = sbP.tile([P, 2, QB], bf16, tag="pT")
        _L(
            nc.scalar.activation(
                pt[:, :, qoff:QB], st_[:, :, qoff:QB], AF.Exp, scale=0.125
            ),
            f"exp_jc{jc}_pr{pr}_kt{kt}",
        )
        return pt, qoff

    def emit_pv(jc, pr, kt, nkt, oext, pt, qoff):
        PH[0] = f"PV_jc{jc}_pr{pr}_kt{kt}"
        for hi in range(2):
            h = pr * 2 + hi
            _MM(
                oext[hi][:, qoff:QB],
                vE[:, kt, h, :],
                pt[:, hi, qoff:QB],
                start=(kt == 0),
                stop=(kt == nkt - 1),
            )

    def emit_norm(jc, pr, oexts):
        # both recips, then both broadcasts, then both mults, so the
        # DVE and Pool chains for the two heads overlap
        q0 = jc * QB
        rcs, rss = [], []
        for hi in range(2):
            rc = sb_norm.tile([1, QB], f32, tag=f"rc{hi}", name=f"rc{hi}")
            nc.vector.reciprocal(rc[:], oexts[pr][hi][HD : HD + 1, :])
            rcs.append(rc)
        for hi in range(2):
            rs = sb_norm.tile([HD, QB], f32, tag=f"rs{hi}", name=f"rs{hi}")
            nc.gpsimd.partition_broadcast(rs[:], rcs[hi][:], channels=HD)
            rss.append(rs)
        for hi in range(2):
            nc.vector.tensor_tensor(
                outT[pr][hi * HD : (hi + 1) * HD, q0 : q0 + QB],
                oexts[pr][hi][0:HD, :],
                rss[hi][:],
                mybir.AluOpType.mult,
            )

    def oext_pair():
        return [
            po.tile([HD + 1, QB], f32, tag="po", name=f"oext{hi}")
            for hi in range(2)
        ]

    return dict(
        qk=emit_qk, v=emit_v, y=emit_y, s=emit_s, pv=emit_pv,
        norm=emit_norm, oext=oext_pair,
    )


def _body(nc, NQB, E, dma_x_chunk):
    """One timing-loop iteration. Chunk 0's x / Q / K / V for the NEXT
    iteration are produced at the end, interleaved with the last chunk's
    output projection."""
    for jc in range(1, NQB):
        dma_x_chunk(jc)

    oexts = {}
    for jc in range(NQB):
        sched = _Sched()
        # this chunk's own K/V, first consumed at the diagonal (kt=4jc)
        if jc >= 1:
            for pr in range(NPAIR):
                sched.add(
                    QK_NS,
                    lambda jc=jc, pr=pr: E["qk"](jc, pr, "k"),
                    deadline=4 * jc - 4,
                )
            for ol in range(QB // P):
                sched.add(
                    V_NS,
                    lambda jc=jc, ol=ol: E["v"](jc, ol),
                    deadline=4 * jc - 2,
                )
        # next chunk's Q, consumed from its first tile
        if jc + 1 < NQB:
            for pr in range(NPAIR):
                sched.add(QK_NS, lambda jc=jc, pr=pr: E["qk"](jc + 1, pr, "q"))
        # previous chunk's output projection
        if jc >= 1:
            for ol in range(QB // P):
                for doc in range(C // QB):
                    sched.add(
                        Y_NS,
                        lambda jc=jc, ol=ol, doc=doc: E["y"](jc - 1, ol, doc),
                    )

        nkt = 4 * jc + 4
        ntiles = NPAIR * nkt
        quota = sum(d for d, _, _ in sched.q) / ntiles
        tile_i = 0

        for pr in range(NPAIR):
            oexts[pr] = E["oext"]()
            # 2-deep software pipeline: PV of tile k goes out after the
            # S of tile k+2, so the exp latency never stalls the PE
            pend = []
            for kt in range(nkt):
                pt, qoff = E["s"](jc, pr, kt)
                pend.append((kt, pt, qoff))
                if kt < 2:
                    # extra pacing while the exp pipeline fills and the
                    # previous pair's norm drains the oext ring
                    sched.pace(quota, tile_i)
                if len(pend) > 2:
                    if kt == 2:
                        sched.pace(quota, tile_i)
                    k0, p0, o0 = pend.pop(0)
                    E["pv"](jc, pr, k0, nkt, oexts[pr], p0, o0)
                    sched.pace(quota, tile_i)
                tile_i += 1
            if jc == NQB - 1 and pr == NPAIR - 1:
                # keep the very tail tight: nothing between the last
                # PVs and the norm that gates the final y projection
                sched.drain()
            for k0, p0, o0 in pend:
                E["pv"](jc, pr, k0, nkt, oexts[pr], p0, o0)
            if pr == NPAIR - 1:
                # drain leftovers before the last norm so their copies
                # are not queued behind the norm chain on DVE/Pool
                sched.drain()
            E["norm"](jc, pr, oexts)

    # tail: the last chunk's output projection interleaved with the NEXT
    # iteration's chunk-0 projections — the projections have no pending
    # dependencies, so the PE stays busy through the final norm latency
    nxt = []
    for pr in range(NPAIR):
        nxt.append(lambda pr=pr: E["qk"](0, pr, "q", ring="st"))
        nxt.append(lambda pr=pr: E["qk"](0, pr, "k", ring="st"))
    for ol in range(QB // P):
        nxt.append(lambda ol=ol: E["v"](0, ol, ring="st"))
    yu = [(ol, doc) for ol in range(QB // P) for doc in range(C // QB)]
    for i in range(max(len(nxt), len(yu))):
        if i < len(nxt):
            nxt[i]()
        if i < len(yu):
            ol, doc = yu[i]
            E["y"](NQB - 1, ol, doc, ring="pp", copy_eng="act")

    # prefetch x chunk 0 for the next iteration (after its last readers)
    dma_x_chunk(0)


def build_core_kernel(nc, tc, T, iters=1):
    TO = T // P  # t-tiles
    NQB = T // QB  # q-blocks / chunks

    xt_d = nc.dram_tensor("xt", [C, T], bf16, kind="ExternalInput").ap()
    wq_d = nc.dram_tensor("wq", [C, DS], bf16, kind="ExternalInput").ap()
    wk_d = nc.dram_tensor("wk", [C, DS], bf16, kind="ExternalInput").ap()
    wv_d = nc.dram_tensor("wv", [C, DS], bf16, kind="ExternalInput").ap()
    wo_d = nc.dram_tensor("wo", [DS, C], bf16, kind="ExternalInput").ap()
    negi_d = nc.dram_tensor("negi", [P, P], bf16, kind="ExternalInput").ap()
    utri_d = nc.dram_tensor("utri", [P, P], bf16, kind="ExternalInput").ap()
    y_d = nc.dram_tensor("y", [T, C], f32, kind="ExternalOutput").ap()

    import contextlib

    stack = contextlib.ExitStack()
    persist = stack.enter_context(tc.tile_pool(name="persist", bufs=1))

    negi = persist.tile([P, P], bf16, tag="negi")
    utri = persist.tile([P, P], bf16, tag="utri")
    nc.sync.dma_start(negi[:], negi_d)
    nc.sync.dma_start(utri[:], utri_d)

    wqT = persist.tile([P, CS, DS], bf16, tag="wqT")
    wkT = persist.tile([P, CS, DS], bf16, tag="wkT")
    wvT = persist.tile([P, CS, DS], bf16, tag="wvT")
    for w_d, w_t in ((wq_d, wqT), (wk_d, wkT), (wv_d, wvT)):
        nc.sync.dma_start(w_t[:], w_d.rearrange("(o p) d -> p o d", p=P))
    woT = persist.tile([P, NPAIR, C], bf16, tag="woT")
    nc.sync.dma_start(woT[:], wo_d.rearrange("(o p) d -> p o d", p=P))

    xT = persist.tile([P, CS, T], bf16, tag="xT")
    qT = [persist.tile([P, T], bf16, tag=f"qT{p}", name=f"qT{p}") for p in range(NPAIR)]
    kT = [persist.tile([P, T], bf16, tag=f"kT{p}", name=f"kT{p}") for p in range(NPAIR)]
    vE = persist.tile([P, TO, HPC, HD + 1], bf16, tag="vE")
    outT = [
        persist.tile([P, T], bf16, tag=f"outT{p}", name=f"outT{p}")
        for p in range(NPAIR)
    ]
    nc.gpsimd.memset(vE[:, :, :, HD : HD + 1], 1.0)

    def dma_x_chunk(jc):
        q0 = jc * QB
        nc.sync.dma_start(
            xT[:, :, q0 : q0 + QB],
            xt_d[:, q0 : q0 + QB].rearrange("(o p) t -> p o t", p=P),
        )

    tl = dict(
        negi=negi, utri=utri, wqT=wqT, wkT=wkT, wvT=wvT, woT=woT,
        xT=xT, qT=qT, kT=kT, vE=vE, outT=outT,
    )

    pools = (
        stack.enter_context(tc.tile_pool(name="st", bufs=2, space="PSUM")),
        stack.enter_context(tc.tile_pool(name="pp", bufs=1, space="PSUM")),
        stack.enter_context(tc.tile_pool(name="po", bufs=3, space="PSUM")),
        stack.enter_context(tc.tile_pool(name="sbP", bufs=4)),
        stack.enter_context(tc.tile_pool(name="sb_norm", bufs=2)),
        stack.enter_context(tc.tile_pool(name="sb_y", bufs=4)),
    )
    E = _make_emitters(nc, tl, pools, y_d)

    # preamble: prime chunk 0 (x + projections) for the first iteration
    dma_x_chunk(0)
    for pr in range(NPAIR):
        E["qk"](0, pr, "q", ring="st")
        E["qk"](0, pr, "k", ring="st")
    for ol in range(QB // P):
        E["v"](0, ol, ring="st")

    if iters < 0:
        # profiling aid: python-unrolled bodies (no hardware loop)
        for _ in range(-iters):
            _body(nc, NQB, E, dma_x_chunk)
    else:
        loop_cm = (
            tc.For_i(0, iters, 1) if iters > 1 else contextlib.nullcontext()
        )
        with loop_cm:
            _body(nc, NQB, E, dma_x_chunk)

    stack.close()


def build_nc(T=T_FULL, iters=1):
    nc = bacc.Bacc("TRN2", target_bir_lowering=False, debug=False, num_devices=N_CORES)
    with tile.TileContext(nc) as tc:
        build_core_kernel(nc, tc, T, iters=iters)
    nc.compile()
    return nc


def make_consts():
    import ml_dtypes

    negi = (-240.0 * np.eye(P)).astype(ml_dtypes.bfloat16)
    k = np.arange(P)
    # utri[k, q] = 1 where masked (q < k) in S^T[k, q]
    utri = (k[None, :] < k[:, None]).astype(ml_dtypes.bfloat16)
    return negi, utri


def make_in_maps(x, Wq, Wk, Wv, Wo):
    """Per-core input dicts. Core c: batch c//4, head group c%4."""
    import ml_dtypes

    bf = ml_dtypes.bfloat16
    negi, utri = make_consts()
    in_maps = []
    for c in range(N_CORES):
        b, g = divmod(c, 4)
        ds = slice(g * DS, (g + 1) * DS)
        in_maps.append(
            {
                "xt": np.ascontiguousarray(x[b].T).astype(bf),
                "wq": np.ascontiguousarray(Wq[ds, :].T).astype(bf),
                "wk": np.ascontiguousarray(Wk[ds, :].T).astype(bf),
                "wv": np.ascontiguousarray(Wv[ds, :].T).astype(bf),
                "wo": np.ascontiguousarray(Wo[:, ds].T).astype(bf),
                "negi": negi,
                "utri": utri,
            }
        )
    return in_maps


def gather(results, bo):
    """Sum partial outputs per batch, add bias."""
    B = N_CORES // 4
    y = np.zeros((B, T_FULL, C), dtype=np.float64)
    for c in range(N_CORES):
        y[c // 4] += np.asarray(results[c]["y"], dtype=np.float64)
    y += bo.astype(np.float64)
    return y.astype(np.float32)


_NC_CACHE = {}


def get_nc():
    if "nc" not in _NC_CACHE:
        _NC_CACHE["nc"] = build_nc()
    return _NC_CACHE["nc"]


def kernel(x, Wq, Wk, Wv, Wo, bo):
    x = np.asarray(x, dtype=np.float32)
    Wq = np.asarray(Wq, dtype=np.float32)
    Wk = np.asarray(Wk, dtype=np.float32)
    Wv = np.asarray(Wv, dtype=np.float32)
    Wo = np.asarray(Wo, dtype=np.float32)
    bo = np.asarray(bo, dtype=np.float32)
    nc = get_nc()
    in_maps = make_in_maps(x, Wq, Wk, Wv, Wo)
    res = run_bass_kernel_spmd(nc, in_maps, core_ids=list(range(N_CORES)))
    return gather(res.results, bo)
